# revision 1
# baseline (speedup 1.0000x reference)
"""Trainium2 Bass kernel for DGCRNNCell (nn_DGCRNNCell_21792664060192).

Computes, for each batch item b and head h over graph with N=199 nodes:
  feat   = einsum('nf,nm->mf', X[b], A*W[h])          (via featT = X^T-style chain)
  dense  = feat @ kernel[h] + bias1[h]
  mask   = softmax(dense - NEG*(1-A), axis=-1)        (adjacency-masked softmax)
  node   = mask @ X[b]
  out_h  = node @ T[h] + bias2[h]
  output[b] = concat([out_0..out_3 (r, 256)], mask_3 (r, 199))   -> (199, 455)

Sharding: pure data-parallel over batch (512 -> 64 per core x 8 cores).

Dataflow on device (per b), all matmul chains arranged so that no transpose
is ever needed (the contraction axis always lands on partitions):
  featT  (f=64, m)    = sum_n  Xb(n,f)^T ... lhsT=Xb chunk, rhs=AW[h] chunk
  denseT (c, r)       : lhsT=kernel[h](f,c-chunk), rhs=featT(f,r)
  expT   (c, r)       = exp(denseT) * EA[h]   where EA[h][c,r] = A[r,c]*e^{bias1[h,c]}
  nodeUT (65, r)      : lhsT=Xb_aug(c,65) (ones col -> row 64 = softmax denom s[r])
  outU   (r, 65h+j)   : lhsT=nodeUT(65, r-chunk), rhs=T_aug[h](65,65)
                        col 64 of each head block = s[r]; row 64 of T_aug = bias2
  out    (r, h*64+u)  = outU * (1/s[r])   (per-partition broadcast multiply)
Head-3 mask output is computed in (r, c) orientation directly:
  dense_rc: lhsT=featT_3(f, r-chunk), rhs=kernel[3](f, c); exp; * EAT3; * 1/s3.
"""

import numpy as np

import concourse.bass as bass
import concourse.mybir as mybir
import concourse.tile as tile
from concourse import bacc
from concourse.bass_utils import run_bass_kernel_spmd

B, N, F, U, H = 512, 199, 64, 64, 4
NCORES = 8
BPC = B // NCORES  # 64 batch items per core
P0 = 128
P1 = N - P0  # 71
FA = F + 1  # X augmented with ones column
OUTC = H * U + N  # 455
DT = mybir.dt.float32
BF = mybir.dt.bfloat16
AF = mybir.ActivationFunctionType
ALU = mybir.AluOpType

_CHUNKS = ((0, P0), (P0, P1))  # (offset, size) along the N(=c or r) axis


def _build_kernel(nc: bass.Bass, tc: "tile.TileContext", io: dict, bpc: int = BPC):
    import os
    from contextlib import ExitStack

    stage = int(os.environ.get("KSTAGE", "9"))

    Xa, XaT, AWc, K2, EAc, TA, ID, O = (
        io["Xa"], io["XaT"], io["AWc"], io["K2"], io["EAc"], io["TA"],
        io["ID"], io["O"],
    )
    fuse = os.environ.get("FUSE45", "0") == "1"

    def _b(name, default):
        return int(os.environ.get(name, str(default)))

    with ExitStack() as ctx:
        cpool = ctx.enter_context(tc.tile_pool(name="consts", bufs=1))
        xpool = ctx.enter_context(tc.tile_pool(name="xa", bufs=_b("XB", 4)))
        fspool = ctx.enter_context(tc.tile_pool(name="fs", bufs=_b("FSB", 3)))
        epool = ctx.enter_context(tc.tile_pool(name="expT", bufs=_b("EB", 3)))
        nspool = ctx.enter_context(tc.tile_pool(name="nS", bufs=_b("NSB", 3)))
        rpool = ctx.enter_context(tc.tile_pool(name="rec", bufs=_b("RB", 4)))
        opool = ctx.enter_context(tc.tile_pool(name="sO", bufs=_b("OB", 4)))
        pf = ctx.enter_context(
            tc.tile_pool(name="pfnu", bufs=_b("FNB", 2), space="PSUM")
        )
        pd = ctx.enter_context(
            tc.tile_pool(name="pdnu", bufs=_b("DTB", 2), space="PSUM")
        )
        po = ctx.enter_context(
            tc.tile_pool(name="poU", bufs=_b("POB", 1), space="PSUM")
        )

        # ---- constants into SBUF (once) ----
        skipc = os.environ.get("SKIPC", "0") == "1"  # timing ablation
        cAW = []
        cEA = []
        for ci, (co, cn) in enumerate(_CHUNKS):
            t = cpool.tile([cn, H, N], BF, name=f"cAW{ci}")
            if not skipc:
                nc.sync.dma_start(t[:], AWc[co : co + cn])
            cAW.append(t)
            t = cpool.tile([cn, 2, 2 * N], BF, name=f"cEA{ci}")
            if not skipc:
                nc.sync.dma_start(t[:], EAc[co : co + cn])
            cEA.append(t)
        cK2 = cpool.tile([128, H, N], BF, name="cK2")
        cTA = cpool.tile([FA, H, FA], BF, name="cTA")
        cID = cpool.tile([128, 128], BF, name="cID")
        if not skipc:
            nc.sync.dma_start(cK2[:], K2[:])
            nc.sync.dma_start(cTA[:], TA[:])
            nc.sync.dma_start(cID[:], ID[:])

        # ---- per batch item ----
        BG = min(_b("BG", 8), bpc)   # input DMA batching
        OG = min(_b("OG", 4), bpc)   # output DMA batching
        xg = [None, None]
        sog = [None, None]
        for b in range(bpc):
            # Group-load BG items of X_aug per chunk in one DMA. Tile layout
            # (cn, FA + BG*FA): cols [FA + g*FA, FA + (g+1)*FA) hold item g's
            # [features | ones]; the leading FA cols are zeroed so the M=128
            # "high-half" lhsT window (64 don't-care cols before the features)
            # is in-bounds for g=0.
            if b % BG == 0:
                ng = min(BG, bpc - b)
                src = Xa[b : b + ng].rearrange("g n f -> n g f")
                xg = []
                for ci, (co, cn) in enumerate(_CHUNKS):
                    t = xpool.tile([cn, BG * FA], BF, tag=f"xa{ci}")
                    if os.environ.get("SKIPX", "0") != "1":  # timing ablation
                        nc.sync.dma_start(
                            t[:, 0 : ng * FA].rearrange("n (g f) -> n g f", f=FA),
                            src[co : co + cn],
                        )
                    xg.append(t)
                if fuse:
                    xtg = xpool.tile([FA, BG * N], BF, tag="xat")
                    if os.environ.get("SKIPX", "0") != "1":
                        nc.sync.dma_start(
                            xtg[:, 0 : ng * N].rearrange("j (g n) -> j g n", n=N),
                            XaT[b : b + ng].rearrange("g j n -> j g n"),
                        )
            g = b % BG
            xa = [t[:, g * FA : (g + 1) * FA] for t in xg]
            # xa[ci] is a (cn, 65) window: [f0..f63 | ones]
            #   step1/2 lhsT -> xa[:, 0:64]; step4 lhsT -> xa[:, 0:65]

            # step1: featT (f, m) for 4 heads packed into one PSUM tile
            # layout [128, 2, N]: partition half = h%2 (0-63 even h, 64-127
            # odd), free slot = h//2. Each head is an M=64 matmul; the
            # even/odd pair targets disjoint PE column groups
            # (tile_position col 0 / 64) so the pair runs concurrently.
            fAB = pf.tile([128, 2, 256], DT, tag="fp")
            for hp in range(2):
                for ci, (co, cn) in enumerate(_CHUNKS):
                    for h in (2 * hp, 2 * hp + 1):
                        pr = 64 * (h % 2)
                        nc.tensor.matmul(
                            fAB[pr : pr + 64, h // 2, 0:N],
                            lhsT=xa[ci][:, 0:64],
                            rhs=cAW[ci][:, h, :],
                            start=(ci == 0),
                            stop=(ci == 1),
                            tile_position=(0, pr),
                        )
            fs = fspool.tile([128, 2, N], BF, tag="fs")
            fce = os.environ.get("FCE", "1")
            if fce == "alt":
                fce = "1" if b % 2 == 0 else "0"
            if fce == "1":
                nc.scalar.copy(fs[:], fAB[:, :, 0:N])
            else:
                nc.vector.tensor_copy(fs[:], fAB[:, :, 0:N])
            if stage <= 1:
                for ci, (ro, rn) in enumerate(_CHUNKS):
                    sO = opool.tile([rn, OUTC], DT, tag=f"sO{ci}")
                    nc.vector.memset(sO[:], 0.0)
                    nc.vector.tensor_copy(sO[:, 0:398], fs[0:rn, :, 0:199])
                    nc.sync.dma_start(O[b, ro : ro + rn], sO[:])
                continue

            # step2: denseT (c, r) -- kernel[h] stationary; h pairs share the
            # PE via disjoint row groups (even h rows 0-63, odd h rows 64-127)
            # head h -> PSUM slot s=h%2 (=bank, so the row-split pair h0/h1
            # writes disjoint banks and may run concurrently), col block k=h//2.
            # The adjacency mask (-1e16 where A=0) plus bias1 is accumulated
            # into the same PSUM region first via an identity-weight matmul
            # streaming the precomputed MK constant, so exp's output is the
            # final masked e with no elementwise fixup pass.
            eT = []
            maskoff = os.environ.get("MASKOFF", "0") == "1"  # timing ablation
            for ci, (co, cn) in enumerate(_CHUNKS):
                t = pd.tile([cn, 2, 512], DT, tag="dnu", name=f"dT{ci}")
                if not maskoff:
                    for sl in range(2):
                        nc.tensor.matmul(
                            t[:, sl, 0 : 2 * N],
                            lhsT=cID[0:cn, 0:cn],
                            rhs=cEA[ci][:, sl, :],
                            start=True,
                            stop=False,
                        )
                for h in range(H):
                    pr = 64 * (h % 2)
                    nc.tensor.matmul(
                        t[:, h % 2, 199 * (h // 2) : 199 * (h // 2) + N],
                        lhsT=cK2[pr : pr + 64, h, co : co + cn],
                        rhs=fs[pr : pr + 64, h // 2, :],
                        start=maskoff,
                        stop=True,
                        tile_position=(pr, 0),
                    )
                e = epool.tile([cn, 2, 2 * N], BF, tag=f"eT{ci}")
                eT.append(e)
                if os.environ.get("ESPLIT", "0") == "1":
                    for sl in range(2):
                        nc.scalar.activation(
                            e[:, sl, :], t[:, sl, 0 : 2 * N], AF.Exp
                        )
                else:
                    nc.scalar.activation(e[:], t[:, :, 0 : 2 * N], AF.Exp)
            if stage <= 2:
                for ci, (ro, rn) in enumerate(_CHUNKS):
                    sO = opool.tile([rn, OUTC], DT, tag=f"sO{ci}")
                    nc.vector.tensor_copy(sO[:, 0:398], eT[ci][:, 0, :])
                    nc.vector.tensor_copy(sO[:, 398:OUTC], eT[ci][:, 1, 0:57])
                    nc.sync.dma_start(O[b, ro : ro + rn], sO[:])
                continue

            if fuse:
                # XT = Xa_aug @ TA_aug per (chunk, head): (cn, 65h+j); col 64
                # of each head block = ones -> s column; TA row 64 = bias2.
                # Off the critical ring: needs only the XaT input + consts.
                xt = xtg[:, g * N : (g + 1) * N]
                XT = pd.tile([128, 2, 512], DT, tag="dnu", name="XT")
                for ci, (co, cn) in enumerate(_CHUNKS):
                    for h in range(H):
                        nc.tensor.matmul(
                            XT[0:cn, ci, 65 * h : 65 * h + 65],
                            lhsT=xt[:, co : co + cn],
                            rhs=cTA[:, h, :],
                            start=True,
                            stop=True,
                        )
                cXT = nspool.tile([128, 2, 260], BF, tag="nS")
                nc.vector.tensor_copy(cXT[:], XT[:, :, 0:260])
                nS = None
            else:
                # step4: nodeUT (65, r) + denominator row via ones col
                nU = pd.tile([65, 2, 512], DT, tag="dnu", name="nU")
                for sl in range(2):
                    for ci, (co, cn) in enumerate(_CHUNKS):
                        nc.tensor.matmul(
                            nU[:, sl, 0 : 2 * N],
                            lhsT=xa[ci][:, 0:65],
                            rhs=eT[ci][:, sl, :],
                            start=(ci == 0),
                            stop=(ci == 1),
                        )
                nS = nspool.tile([65, 2, 2 * N], BF, tag="nS")
                nc.vector.tensor_copy(nS[:], nU[:, :, 0 : 2 * N])
            if stage <= 3:
                for ci, (ro, rn) in enumerate(_CHUNKS):
                    sO = opool.tile([rn, OUTC], DT, tag=f"sO{ci}")
                    nc.vector.memset(sO[:], 0.0)
                    nc.sync.dma_start(O[b, ro : ro + rn], sO[:])
                continue

            # head-3 mask transposed into (r, c) orientation for the output:
            # PE-transpose of the already-masked e3 (slot 1, col block 1)
            pR = pf.tile([128, 2, 256], BF, tag="fp")
            for rj, (ro, rn) in enumerate(_CHUNKS):
                for ci, (co, cn) in enumerate(_CHUNKS):
                    nc.tensor.transpose(
                        pR[0:rn, rj, co : co + cn],
                        in_=eT[ci][:, 1, N + ro : N + ro + rn],
                        identity=cID[0:cn, 0:cn],
                    )

            # step5 per r-chunk; outputs staged in OG-item groups and DMA'd
            # out with one descriptor set per group per chunk
            go = b % OG
            if go == 0:
                sog = [
                    opool.tile([rn, OG, OUTC], DT, tag=f"sO{ci}", name=f"sOg{ci}")
                    for ci, (ro, rn) in enumerate(_CHUNKS)
                ]
            for ci, (ro, rn) in enumerate(_CHUNKS):
                oUF = po.tile(
                    [rn, 260], DT,
                    tag="oU" if os.environ.get("OUM", "0") == "1" else f"oUF{ci}",
                    bufs=2 if os.environ.get("OUM", "0") == "1" else None,
                )
                for h in range(H):
                    if fuse:
                        # out_h = (e_h)^T @ XT_h, accumulated over c-chunks
                        for cc, (co, cn) in enumerate(_CHUNKS):
                            nc.tensor.matmul(
                                oUF[:, 65 * h : 65 * h + 65],
                                lhsT=eT[cc][
                                    :, h % 2,
                                    199 * (h // 2) + ro : 199 * (h // 2) + ro + rn,
                                ],
                                rhs=cXT[0:cn, cc, 65 * h : 65 * h + 65],
                                start=(cc == 0),
                                stop=(cc == 1),
                            )
                    else:
                        nc.tensor.matmul(
                            oUF[:, 65 * h : 65 * h + 65],
                            lhsT=nS[
                                :, h % 2,
                                199 * (h // 2) + ro : 199 * (h // 2) + ro + rn,
                            ],
                            rhs=cTA[:, h, :],
                            start=True,
                            stop=True,
                        )

                # 1/s for all 4 heads: s sits at col 64 of each 65-wide block
                rec = rpool.tile([rn, H], DT, tag=f"rec{ci}")
                oUh = oUF[:].rearrange("p (h j) -> p h j", j=65)
                nc.vector.reciprocal(rec[:], oUh[:, :, 64])

                sO = sog[ci][:, go]
                # head outputs normalized by 1/s (free-dim broadcast of rec)
                nc.vector.tensor_tensor(
                    sO[:, 0 : H * U].rearrange("p (h u) -> p h u", u=U),
                    oUh[:, :, 0:U],
                    rec[:, :, None].to_broadcast((rn, H, U)),
                    ALU.mult,
                )
                # head-3 mask: transposed-masked e3, normalized
                nc.vector.tensor_scalar_mul(
                    sO[:, H * U : OUTC], pR[0:rn, ci, 0:N], rec[:, 3:4]
                )

                if go == OG - 1 or b == bpc - 1:
                    ng = go + 1
                    if os.environ.get("SKIPO", "0") != "1":  # timing ablation
                        oq = (
                            nc.gpsimd
                            if os.environ.get("ODMAQ", "sync") == "gpsimd"
                            else nc.sync
                        )
                        oq.dma_start(
                            O[b - go : b + 1, ro : ro + rn].rearrange(
                                "g n c -> n g c"
                            ),
                            sog[ci][:, 0:ng],
                        )


def build_nc(
    bpc: int = BPC, num_devices: int = NCORES, repeat: int = 1
) -> bass.Bass:
    nc = bacc.Bacc(
        "TRN2",
        target_bir_lowering=False,
        debug=False,
        num_devices=num_devices,
    )
    io = {
        "Xa": nc.dram_tensor("Xa", [bpc, N, FA], BF, kind="ExternalInput").ap(),
        "XaT": nc.dram_tensor("XaT", [bpc, FA, N], BF, kind="ExternalInput").ap(),
        "AWc": nc.dram_tensor("AWc", [N, H, N], BF, kind="ExternalInput").ap(),
        "K2": nc.dram_tensor("K2", [128, H, N], BF, kind="ExternalInput").ap(),
        "EAc": nc.dram_tensor("EAc", [N, 2, 2 * N], BF, kind="ExternalInput").ap(),
        "TA": nc.dram_tensor("TA", [FA, H, FA], BF, kind="ExternalInput").ap(),
        "ID": nc.dram_tensor("ID", [128, 128], BF, kind="ExternalInput").ap(),
        "O": nc.dram_tensor("O", [bpc, N, OUTC], DT, kind="ExternalOutput").ap(),
    }
    with tile.TileContext(nc) as tc:
        if repeat == 1:
            _build_kernel(nc, tc, io, bpc=bpc)
        else:
            # Timing-only variant: re-run the identical workload `repeat`
            # times in a hardware loop (same output written each pass) so
            # per-dispatch tunnel latency can be amortized out of the
            # hardware-time measurement. staggered_reset avoids charging the
            # loop's all-engine barrier (a measurement artifact a single
            # dispatch never pays) to every iteration.
            import os as _os

            if _os.environ.get("STAGR", "1") == "1":
                with tc.For_i(0, repeat, 1, staggered_reset=True):
                    _build_kernel(nc, tc, io, bpc=bpc)
            else:
                with tc.For_i(0, repeat, 1):
                    _build_kernel(nc, tc, io, bpc=bpc)
    nc.compile()
    return nc


def _prep_weights(A, W, kernel, T, bias1, bias2):
    """Host-side constant prep (tiny tensors)."""
    A = np.asarray(A, np.float32)
    W = np.asarray(W, np.float32)
    kernel = np.asarray(kernel, np.float32)
    T = np.asarray(T, np.float32)
    bias1 = np.asarray(bias1, np.float32)
    bias2 = np.asarray(bias2, np.float32)

    AW = A[None, :, :] * W  # (H, n, m)
    AWc = np.ascontiguousarray(AW.transpose(1, 0, 2))  # [n, h, m]

    Kf = kernel  # (H, F, N): [h, f, c]
    K1 = np.ascontiguousarray(Kf.transpose(1, 0, 2))  # [f, h, c]
    K2 = np.concatenate([K1, K1], axis=0)  # duplicate f-rows for PE rows 64-127

    # MK[c, h, r] = bias1[h, c] - 1e16 * (1 - A[r, c]): additive logit fixup
    # (adjacency mask + bias1) accumulated into dense via identity matmul;
    # packed as [c, s, k*199 + r] with h = 2k + s (s = slot/bank, k = block)
    MK = bias1.T[:, :, None] - 1e16 * (1.0 - A.T[:, None, :])  # (c, h, r)
    EAc = np.ascontiguousarray(
        MK.reshape(N, 2, 2, N).transpose(0, 2, 1, 3).reshape(N, 2, 2 * N)
    )

    # T_aug2[h]: (65, 65): rows 0-63 = T[h], row 64 = [bias2[h], 1.0-at-col-64]
    TA = np.zeros((FA, H, FA), np.float32)
    TA[:F, :, :U] = T.transpose(1, 0, 2)
    TA[F, :, :U] = bias2
    TA[F, :, U] = 1.0
    import ml_dtypes

    bf = ml_dtypes.bfloat16
    return dict(
        AWc=AWc.astype(bf), K2=K2.astype(bf), EAc=EAc.astype(bf),
        TA=TA.astype(bf), ID=np.eye(128, dtype=bf),
    )


_CACHED = {}


def _get_executable(repeat: int = 1):
    """Build the Bass module once and wrap it in a reusable sharded jax jit.

    Mirrors concourse.bass2jax.run_bass_via_pjrt's multi-core path, but caches
    the jitted callable so repeated kernel() calls skip re-lowering the BIR.
    """
    if repeat in _CACHED:
        return _CACHED[repeat]

    import jax
    from jax.sharding import Mesh, PartitionSpec
    from jax.experimental.shard_map import shard_map

    import concourse.mybir as _mybir
    from concourse import bass2jax

    bass2jax.install_neuronx_cc_hook()
    nc = build_nc(repeat=repeat)

    partition_name = (
        nc.partition_id_tensor.name if nc.partition_id_tensor else None
    )
    in_names, out_names, out_avals = [], [], []
    for alloc in nc.m.functions[0].allocations:
        if not isinstance(alloc, _mybir.MemoryLocationSet):
            continue
        name = alloc.memorylocations[0].name
        if alloc.kind == "ExternalInput":
            if name != partition_name:
                in_names.append(name)
        elif alloc.kind == "ExternalOutput":
            out_names.append(name)
            out_avals.append(
                jax.core.ShapedArray(
                    tuple(alloc.tensor_shape), _mybir.dt.np(alloc.dtype)
                )
            )
    n_params = len(in_names)
    n_outs = len(out_avals)
    all_in_names = list(in_names) + list(out_names)
    if partition_name is not None:
        all_in_names.append(partition_name)

    def _body(*args):
        operands = list(args)
        if partition_name is not None:
            operands.append(bass2jax.partition_id_tensor())
        outs = bass2jax._bass_exec_p.bind(
            *operands,
            out_avals=tuple(out_avals),
            in_names=tuple(all_in_names),
            out_names=tuple(out_names),
            lowering_input_output_aliases=(),
            sim_require_finite=True,
            sim_require_nnan=True,
            nc=nc,
        )
        return tuple(outs)

    devices = jax.devices()[:NCORES]
    mesh = Mesh(np.asarray(devices), ("core",))
    in_specs = (PartitionSpec("core"),) * (n_params + n_outs)
    out_specs = (PartitionSpec("core"),) * n_outs
    sharded = jax.jit(
        shard_map(
            _body, mesh=mesh, in_specs=in_specs, out_specs=out_specs,
            check_rep=False,
        ),
        donate_argnums=tuple(range(n_params, n_params + n_outs)),
        keep_unused=True,
    )
    _CACHED[repeat] = (sharded, in_names, out_names, out_avals, jax, mesh)
    return _CACHED[repeat]


def _stage_inputs(inputs):
    import ml_dtypes

    X = np.asarray(inputs["X"], np.float32)
    consts = _prep_weights(
        inputs["A"], inputs["W"], inputs["kernel"], inputs["T"],
        inputs["bias1"], inputs["bias2"],
    )
    Xa = np.concatenate([X, np.ones((B, N, 1), np.float32)], axis=2)
    Xab = Xa.astype(ml_dtypes.bfloat16)
    # (B, N, FA) and its per-item transpose (B, FA, N): axis0 shards
    per_core = {
        "Xa": np.ascontiguousarray(Xab),
        "XaT": np.ascontiguousarray(Xab.transpose(0, 2, 1)),
    }
    for k, v in consts.items():
        per_core[k] = np.concatenate([v] * NCORES, axis=0)
    return per_core


def _run(staged):
    sharded, in_names, out_names, out_avals, jax, mesh = _get_executable()
    concat_in = [staged[nm] for nm in in_names]
    zeros = [
        np.zeros((NCORES * a.shape[0], *a.shape[1:]), a.dtype) for a in out_avals
    ]
    out_arrs = sharded(*concat_in, *zeros)
    return np.asarray(out_arrs[out_names.index("O")])


def kernel(**inputs) -> np.ndarray:
    staged = _stage_inputs(inputs)
    out = _run(staged)  # (NCORES*BPC, N, OUTC) = (B, N, OUTC)
    return out



# revision 11
# speedup vs baseline: 1.3516x; 1.3516x over previous
"""Trainium2 Bass kernel for DGCRNNCell (nn_DGCRNNCell_21792664060192).

Computes, for each batch item b and head h over graph with N=199 nodes:
  feat   = einsum('nf,nm->mf', X[b], A*W[h])          (via featT chain)
  dense  = feat @ kernel[h] + bias1[h]
  mask   = softmax(dense - NEG*(1-A), axis=-1)        (adjacency-masked softmax)
  node   = mask @ X[b]
  out_h  = node @ T[h] + bias2[h]
  output[b] = concat([out_0..out_3 (r, 256)], mask_3 (r, 199))   -> (199, 455)

Sharding: pure data-parallel over batch (512 -> 64 per core x 8 cores).

v3 dataflow (per core), built around item PAIRS and engine balance
(GPSIMD cannot touch PSUM on TRN2, so all PSUM-side elementwise work is
split between ACT and DVE with as few, as large instructions as possible):
  step1  featT for a pair (b0,b1): lhsT = [X[b0] | X[b1]] (cn, 128) so the
         pair's f-rows land on partitions 0-63 / 64-127; rhs = AW head-pair
         (cn, 2*199).  4 matmuls of free 398 per pair.
  fs     PSUM->SBUF bf16 copy of the pair's featT; alternates ACT/DVE.
  dense  per item: adjacency mask + bias1 written via an fp8 DoubleRow
         identity matmul (half cycles; -60/0 are exact in fp8e4), then 8
         bf16 matmuls accumulate kernel[h]^T @ featT; exp on ACT gives the
         masked e directly (one activation per c-chunk).
  XT     per item: X_aug @ T_aug, ONE matmul per c-chunk (free 260);
         col 64 of each head block = ones -> s; TA row 64 = bias2.
  step5  out = (e_h)^T @ XT_h accumulated over c-chunks into a 2-item PSUM
         tile; head-3 mask via PE transpose of e3 into a 2-item PSUM tile.
  stage  per item pair: ONE reciprocal, ONE normalize-multiply (512 free)
         and ONE mask3-multiply (398 free) on DVE; output staged bf16
         (host casts to fp32).
PSUM budget (8 banks): ring{fAB,d0,d1,XT} 2x2 banks; oUF2 2 banks;
pR2 2x1 banks.
"""

import numpy as np

import concourse.bass as bass
import concourse.mybir as mybir
import concourse.tile as tile
from concourse import bacc

B, N, F, U, H = 512, 199, 64, 64, 4
NCORES = 8
BPC = B // NCORES  # 64 batch items per core
P0 = 128
P1 = N - P0  # 71
FA = F + 1  # X augmented with ones column (XaT row 64 = ones)
OUTC = H * U + N  # 455
DT = mybir.dt.float32
BF = mybir.dt.bfloat16
F8 = mybir.dt.float8e4
AF = mybir.ActivationFunctionType
ALU = mybir.AluOpType
PM = mybir.MatmulPerfMode

_CHUNKS = ((0, P0), (P0, P1))  # (offset, size) along the N(=c or r) axis


def _build_kernel_v3(nc: bass.Bass, tc: "tile.TileContext", io: dict, bpc: int = BPC):
    import os
    from contextlib import ExitStack

    Xf, XaT, AWp, K2, MK8, ID8, TA, ID, O = (
        io["Xf"], io["XaT"], io["AWp"], io["K2"], io["MK8"], io["ID8"],
        io["TA"], io["ID"], io["O"],
    )

    def _b(name, default):
        return int(os.environ.get(name, str(default)))

    fse = os.environ.get("FSE", "alt")     # fs copy engine: alt|scalar|vector
    mask8 = os.environ.get("MASK8", "1") == "1"  # fp8 DoubleRow mask write

    with ExitStack() as ctx:
        cpool = ctx.enter_context(tc.tile_pool(name="consts", bufs=1))
        xpool = ctx.enter_context(tc.tile_pool(name="xf", bufs=_b("XB", 2)))
        fspool = ctx.enter_context(tc.tile_pool(name="fs", bufs=_b("FSB", 2)))
        epool = ctx.enter_context(tc.tile_pool(name="expT", bufs=_b("EB", 3)))
        rpool = ctx.enter_context(tc.tile_pool(name="rec", bufs=_b("RB", 4)))
        opool = ctx.enter_context(tc.tile_pool(name="sO", bufs=_b("OB", 2)))

        # ---- constants into SBUF (once) ----
        cAW = []
        cMK = []
        for ci, (co, cn) in enumerate(_CHUNKS):
            t = cpool.tile([cn, 2, 2 * N], BF, name=f"cAW{ci}")
            nc.sync.dma_start(t[:], AWp[co : co + cn])
            cAW.append(t)
            if mask8:
                t = cpool.tile([cn, 2, 2, 2 * N], F8, name=f"cMK{ci}")
                nc.sync.dma_start(t[:], MK8[co : co + cn])
                cMK.append(t)
        cK2 = cpool.tile([128, H, N], BF, name="cK2")
        cTA = cpool.tile([FA, H, FA], BF, name="cTA")
        cID = cpool.tile([128, 128], BF, name="cID")
        nc.sync.dma_start(cK2[:], K2[:])
        nc.sync.dma_start(cTA[:], TA[:])
        nc.sync.dma_start(cID[:], ID[:])
        if mask8:
            cID8 = cpool.tile([128, 2, 128], F8, name="cID8")
            nc.sync.dma_start(cID8[:], ID8[:])

        BG = min(_b("BG", 8), bpc)   # input DMA batching
        OG = min(_b("OG", 4), bpc)   # output DMA batching (multiple of 2)

        # ---- prologue: XT = Xa_aug @ TA_aug for ALL items, kept in SBUF ----
        # cXTall[c, b, cc, 65h + j]: cols 0-63 of each head block = XT data,
        # col 64 = ones (the softmax-denominator column), written once.
        cXTall = cpool.tile([128, bpc, 2, H * FA], BF, name="cXTall")
        nc.vector.memset(
            cXTall[:].rearrange("p b c (h j) -> p b c h j", j=FA)[:, :, :, :, 64],
            1.0,
        )
        PG = 4  # items per prologue PSUM tile
        with tc.tile_pool(name="pxt", bufs=_b("XTB", 2), space="PSUM") as pxt:
            for b0 in range(0, bpc, PG):
                if b0 % BG == 0:
                    ng = min(BG, bpc - b0)
                    xtg = xpool.tile([FA, BG * N], BF, tag="xat")
                    nc.sync.dma_start(
                        xtg[:, 0 : ng * N].rearrange("j (g n) -> j g n", n=N),
                        XaT[b0 : b0 + ng].rearrange("g j n -> j g n"),
                    )
                gi = b0 % BG
                XTp = pxt.tile([128, PG, 2, 256], DT, tag="xtp")
                for g in range(PG):
                    xt = xtg[:, (gi + g) * N : (gi + g + 1) * N]
                    for ci, (co, cn) in enumerate(_CHUNKS):
                        nc.tensor.matmul(
                            XTp[0:cn, g, ci, :],
                            lhsT=xt[:, co : co + cn],
                            rhs=cTA[:, :, 0:U],
                            start=True,
                            stop=True,
                        )
                dst = cXTall[:, b0 : b0 + PG].rearrange(
                    "p b c (h j) -> p b c h j", j=FA
                )[:, :, :, :, 0:U]
                if (b0 // PG) % 2 == 0:
                    nc.scalar.copy(
                        dst, XTp[:].rearrange("p b c (h u) -> p b c h u", u=U)
                    )
                else:
                    nc.vector.tensor_copy(
                        dst, XTp[:].rearrange("p b c (h u) -> p b c h u", u=U)
                    )

        pd = ctx.enter_context(
            tc.tile_pool(name="pdnu", bufs=_b("DTB", 2), space="PSUM")
        )
        po = ctx.enter_context(
            tc.tile_pool(name="poU", bufs=_b("POB", 2), space="PSUM")
        )

        xg = [None, None]
        sog = [None, None]
        for q in range(bpc // 2):
            b0 = 2 * q
            if b0 % BG == 0:
                ng = min(BG, bpc - b0)
                src = Xf[b0 : b0 + ng].rearrange("g n f -> n g f")
                xg = []
                for ci, (co, cn) in enumerate(_CHUNKS):
                    t = xpool.tile([cn, BG, F], BF, tag=f"xf{ci}")
                    nc.sync.dma_start(t[:, 0:ng, :], src[co : co + cn])
                    xg.append(t)
            gi = b0 % BG

            # step1 for the pair: out partitions = [b0 f | b1 f]
            fAB = pd.tile([128, 2, 512], DT, tag="dnu", name="fAB")
            for hp in range(2):
                for ci, (co, cn) in enumerate(_CHUNKS):
                    nc.tensor.matmul(
                        fAB[:, hp, 0 : 2 * N],
                        lhsT=xg[ci][:, gi : gi + 2, :],
                        rhs=cAW[ci][:, hp, :],
                        start=(ci == 0),
                        stop=(ci == 1),
                    )
            fs = fspool.tile([128, 2, 2 * N], BF, tag="fs")
            use_scalar = fse == "scalar" or (fse == "alt" and q % 2 == 0)
            if use_scalar:
                nc.scalar.copy(fs[:], fAB[:, :, 0 : 2 * N])
            else:
                nc.vector.tensor_copy(fs[:], fAB[:, :, 0 : 2 * N])

            eTg = []   # per item: [eT0, eT1]
            for g in range(2):
                # dense chunks + exp; head h -> slot s=h%2, block k=h//2
                eT = []
                for ci, (co, cn) in enumerate(_CHUNKS):
                    d = pd.tile([128, 2, 512], DT, tag="dnu", name=f"dT{ci}")
                    if mask8:
                        for s in range(2):
                            nc.tensor.matmul(
                                d[0:cn, s, 0 : 2 * N],
                                lhsT=cID8[0:cn, :, 0:cn],
                                rhs=cMK[ci][:, s],
                                start=True,
                                stop=False,
                                perf_mode=PM.DoubleRow,
                            )
                    for h in range(H):
                        nc.tensor.matmul(
                            d[0:cn, h % 2, 199 * (h // 2) : 199 * (h // 2) + N],
                            lhsT=cK2[64 * g : 64 * g + 64, h, co : co + cn],
                            rhs=fs[64 * g : 64 * g + 64, h // 2,
                                   199 * (h % 2) : 199 * (h % 2) + N],
                            start=not mask8,
                            stop=True,
                            tile_position=(64 * g, 0),
                        )
                    e = epool.tile([cn, 2, 2 * N], BF, tag=f"eT{ci}")
                    nc.scalar.activation(e[:], d[0:cn, :, 0 : 2 * N], AF.Exp)
                    eT.append(e)
                eTg.append(eT)

            # step5 per item, interleaved so item 0's transposes + step5
            # matmuls fill the PE while item 1's exp runs.
            go = b0 % OG
            if go == 0:
                sog = [
                    opool.tile([rn, OG, OUTC], BF, tag=f"sO{ci}", name=f"sOg{ci}")
                    for ci, (ro, rn) in enumerate(_CHUNKS)
                ]
            oU2 = [po.tile([128, 2, 512], DT, tag="oU", name=f"oUF{ci}")
                   for ci in range(2)]
            # head-3 mask lives (as bf16) in the padding of the oU2 slots:
            # slot bytes [0:1040) hold the 260-col step5 output, [1040:1840)
            # hold the transposed e3 row-chunk for the same r-range.
            pRv = [oU2[ci][:].bitcast(BF) for ci in range(2)]
            for g in range(2):
                # head-3 mask transposed into (r, c): PE-transpose of e3
                for rj, (ro, rn) in enumerate(_CHUNKS):
                    for ci, (co, cn) in enumerate(_CHUNKS):
                        nc.tensor.transpose(
                            pRv[rj][0:rn, g, 520 + co : 520 + co + cn],
                            in_=eTg[g][ci][:, 1, N + ro : N + ro + rn],
                            identity=cID[0:cn, 0:cn],
                        )
                for ci, (ro, rn) in enumerate(_CHUNKS):
                    for h in range(H):
                        for cc, (co, cn) in enumerate(_CHUNKS):
                            nc.tensor.matmul(
                                oU2[ci][0:rn, g, 65 * h : 65 * h + 65],
                                lhsT=eTg[g][cc][
                                    :, h % 2,
                                    199 * (h // 2) + ro : 199 * (h // 2) + ro + rn,
                                ],
                                rhs=cXTall[0:cn, b0 + g, cc,
                                           65 * h : 65 * h + 65],
                                start=(cc == 0),
                                stop=(cc == 1),
                            )

            for ci, (ro, rn) in enumerate(_CHUNKS):
                oUF2 = oU2[ci]
                # 1/s for both items x 4 heads: s at col 64 of each 65-block
                rec = rpool.tile([rn, 2, H], DT, tag=f"rec{ci}")
                oUh = oUF2[0:rn, :, 0 : H * FA].rearrange(
                    "p g (h j) -> p g h j", j=FA
                )
                nc.vector.reciprocal(rec[:], oUh[:, :, :, 64])

                sO2 = sog[ci][:, go : go + 2]
                nc.vector.tensor_tensor(
                    sO2[:, :, 0 : H * U].rearrange("p g (h u) -> p g h u", u=U),
                    oUh[:, :, :, 0:U],
                    rec[:, :, :, None].to_broadcast((rn, 2, H, U)),
                    ALU.mult,
                )
                nc.vector.tensor_tensor(
                    sO2[:, :, H * U : OUTC],
                    pRv[ci][0:rn, :, 520 : 520 + N],
                    rec[:, :, 3:4].to_broadcast((rn, 2, N)),
                    ALU.mult,
                )

                if go + 2 == OG or b0 + 2 >= bpc:
                    ngo = go + 2
                    nc.sync.dma_start(
                        O[b0 + 1 - (ngo - 1) : b0 + 2, ro : ro + rn].rearrange(
                            "g n c -> n g c"
                        ),
                        sog[ci][:, 0:ngo],
                    )


def build_nc(
    bpc: int = BPC, num_devices: int = NCORES, repeat: int = 1
) -> bass.Bass:
    nc = bacc.Bacc(
        "TRN2",
        target_bir_lowering=False,
        debug=False,
        num_devices=num_devices,
    )
    io = {
        "Xf": nc.dram_tensor("Xf", [bpc, N, F], BF, kind="ExternalInput").ap(),
        "XaT": nc.dram_tensor("XaT", [bpc, FA, N], BF, kind="ExternalInput").ap(),
        "AWp": nc.dram_tensor("AWp", [N, 2, 2 * N], BF, kind="ExternalInput").ap(),
        "K2": nc.dram_tensor("K2", [128, H, N], BF, kind="ExternalInput").ap(),
        "MK8": nc.dram_tensor("MK8", [N, 2, 2, 2 * N], F8, kind="ExternalInput").ap(),
        "ID8": nc.dram_tensor("ID8", [128, 2, 128], F8, kind="ExternalInput").ap(),
        "TA": nc.dram_tensor("TA", [FA, H, FA], BF, kind="ExternalInput").ap(),
        "ID": nc.dram_tensor("ID", [128, 128], BF, kind="ExternalInput").ap(),
        "O": nc.dram_tensor("O", [bpc, N, OUTC], BF, kind="ExternalOutput").ap(),
    }
    with tile.TileContext(nc) as tc:
        if repeat == 1:
            _build_kernel_v3(nc, tc, io, bpc=bpc)
        else:
            # Timing-only variant: re-run the identical workload `repeat`
            # times in a hardware loop so per-dispatch tunnel latency can be
            # amortized out of the hardware-time measurement.
            import os as _os

            if _os.environ.get("STAGR", "1") == "1":
                with tc.For_i(0, repeat, 1, staggered_reset=True):
                    _build_kernel_v3(nc, tc, io, bpc=bpc)
            else:
                with tc.For_i(0, repeat, 1):
                    _build_kernel_v3(nc, tc, io, bpc=bpc)
    nc.compile()
    return nc


def _prep_weights(A, W, kernel, T, bias1, bias2):
    """Host-side constant prep (tiny tensors)."""
    A = np.asarray(A, np.float32)
    W = np.asarray(W, np.float32)
    kernel = np.asarray(kernel, np.float32)
    T = np.asarray(T, np.float32)
    bias1 = np.asarray(bias1, np.float32)
    bias2 = np.asarray(bias2, np.float32)

    AW = A[None, :, :] * W  # (H, n, m)
    # AWp[n, hp, k*199+m] = AW[2hp+k][n, m]
    AWp = np.ascontiguousarray(
        AW.reshape(2, 2, N, N).transpose(2, 0, 1, 3).reshape(N, 2, 2 * N)
    )

    Kf = kernel  # (H, F, N): [h, f, c]
    K1 = np.ascontiguousarray(Kf.transpose(1, 0, 2))  # [f, h, c]
    K2 = np.concatenate([K1, K1], axis=0)  # duplicate f-rows for PE rows 64-127

    # MK[c, h, m] = bias1[h, c] - 60 * (1 - A[m, c]): additive logit fixup
    # (adjacency mask + bias1); -60 and 0 are exactly representable in fp8e4.
    # Packed as [c, s, ktile, k*199 + m] with h = 2k + s; ktile 1 is zeros
    # (the second DoubleRow contraction tile contributes nothing).
    MK = bias1.T[:, :, None] - 60.0 * (1.0 - A.T[:, None, :])  # (c, h, m)
    MKs = MK.reshape(N, 2, 2, N).transpose(0, 2, 1, 3).reshape(N, 2, 2 * N)
    MK8 = np.zeros((N, 2, 2, 2 * N), np.float32)
    MK8[:, :, 0, :] = MKs

    # T_aug[h]: (65, 65): rows 0-63 = T[h], row 64 = [bias2[h], 1.0-at-col-64]
    TA = np.zeros((FA, H, FA), np.float32)
    TA[:F, :, :U] = T.transpose(1, 0, 2)
    TA[F, :, :U] = bias2
    TA[F, :, U] = 1.0

    ID8 = np.zeros((128, 2, 128), np.float32)
    ID8[:, 0, :] = np.eye(128, dtype=np.float32)

    import ml_dtypes

    bf = ml_dtypes.bfloat16
    f8 = ml_dtypes.float8_e4m3
    return dict(
        AWp=AWp.astype(bf), K2=K2.astype(bf), MK8=MK8.astype(f8),
        ID8=ID8.astype(f8), TA=TA.astype(bf), ID=np.eye(128, dtype=bf),
    )


_CACHED = {}


def _get_executable(repeat: int = 1):
    """Build the Bass module once and wrap it in a reusable sharded jax jit.

    Mirrors concourse.bass2jax.run_bass_via_pjrt's multi-core path, but caches
    the jitted callable so repeated kernel() calls skip re-lowering the BIR.
    """
    if repeat in _CACHED:
        return _CACHED[repeat]

    import jax
    from jax.sharding import Mesh, PartitionSpec
    from jax.experimental.shard_map import shard_map

    import concourse.mybir as _mybir
    from concourse import bass2jax

    bass2jax.install_neuronx_cc_hook()
    nc = build_nc(repeat=repeat)

    partition_name = (
        nc.partition_id_tensor.name if nc.partition_id_tensor else None
    )
    in_names, out_names, out_avals = [], [], []
    for alloc in nc.m.functions[0].allocations:
        if not isinstance(alloc, _mybir.MemoryLocationSet):
            continue
        name = alloc.memorylocations[0].name
        if alloc.kind == "ExternalInput":
            if name != partition_name:
                in_names.append(name)
        elif alloc.kind == "ExternalOutput":
            out_names.append(name)
            out_avals.append(
                jax.core.ShapedArray(
                    tuple(alloc.tensor_shape), _mybir.dt.np(alloc.dtype)
                )
            )
    n_params = len(in_names)
    n_outs = len(out_avals)
    all_in_names = list(in_names) + list(out_names)
    if partition_name is not None:
        all_in_names.append(partition_name)

    def _body(*args):
        operands = list(args)
        if partition_name is not None:
            operands.append(bass2jax.partition_id_tensor())
        outs = bass2jax._bass_exec_p.bind(
            *operands,
            out_avals=tuple(out_avals),
            in_names=tuple(all_in_names),
            out_names=tuple(out_names),
            lowering_input_output_aliases=(),
            sim_require_finite=True,
            sim_require_nnan=True,
            nc=nc,
        )
        return tuple(outs)

    devices = jax.devices()[:NCORES]
    mesh = Mesh(np.asarray(devices), ("core",))
    in_specs = (PartitionSpec("core"),) * (n_params + n_outs)
    out_specs = (PartitionSpec("core"),) * n_outs
    sharded = jax.jit(
        shard_map(
            _body, mesh=mesh, in_specs=in_specs, out_specs=out_specs,
            check_rep=False,
        ),
        donate_argnums=tuple(range(n_params, n_params + n_outs)),
        keep_unused=True,
    )
    _CACHED[repeat] = (sharded, in_names, out_names, out_avals, jax, mesh)
    return _CACHED[repeat]


def _stage_inputs(inputs):
    import ml_dtypes

    X = np.asarray(inputs["X"], np.float32)
    consts = _prep_weights(
        inputs["A"], inputs["W"], inputs["kernel"], inputs["T"],
        inputs["bias1"], inputs["bias2"],
    )
    bf = ml_dtypes.bfloat16
    Xb = X.astype(bf)
    XaT = np.concatenate(
        [X.transpose(0, 2, 1), np.ones((B, 1, N), np.float32)], axis=1
    ).astype(bf)
    per_core = {
        "Xf": np.ascontiguousarray(Xb),
        "XaT": np.ascontiguousarray(XaT),
    }
    for k, v in consts.items():
        per_core[k] = np.concatenate([v] * NCORES, axis=0)
    return per_core


def _run(staged):
    sharded, in_names, out_names, out_avals, jax, mesh = _get_executable()
    concat_in = [staged[nm] for nm in in_names]
    zeros = [
        np.zeros((NCORES * a.shape[0], *a.shape[1:]), a.dtype) for a in out_avals
    ]
    out_arrs = sharded(*concat_in, *zeros)
    return np.asarray(out_arrs[out_names.index("O")])


def kernel(**inputs) -> np.ndarray:
    staged = _stage_inputs(inputs)
    out = _run(staged)  # (NCORES*BPC, N, OUTC) = (B, N, OUTC) bf16
    return out.astype(np.float32)
